# revision 11
# baseline (speedup 1.0000x reference)
"""NeuronRouter kernel for 8x Trainium2 NeuronCores.

Problem (hardcoded shapes): B=8, S=1024, D=1024, H=16, DH=64, N=1024, R=32, K=16.

Sharding: data-parallel over batch B — core b computes batch element b.
Everything else (weights, neuron pool) is replicated.

All score-path matmuls run in native fp32 (4 cyc/row) so the top-k selection
matches a pure-fp32 reference exactly; outputs are fp32-accurate.

Structure (v2): s-chunk pipelined. After the projections, each 128-token
s-chunk flows through attention -> path-combine -> scores -> top-16 ->
selected-row gather independently, so the (DMA-heavy) gather of chunk i
overlaps the compute of chunks i+1.. and the serial tail is one chunk long.
"""

import numpy as np

import concourse.bass as bass
import concourse.mybir as mybir
from concourse import bacc
from concourse.tile import TileContext
from concourse.bass_utils import run_bass_kernel_spmd
from concourse.masks import make_identity

F32 = mybir.dt.float32
U32 = mybir.dt.uint32
I32 = mybir.dt.int32
AF = mybir.ActivationFunctionType
ALU = mybir.AluOpType

B, S, D = 8, 1024, 1024
H, DH = 16, 64
N, R, K = 1024, 32, 16
P = 128
NCORES = 8

_CACHE = {}


def _build():
    nc = bacc.Bacc(target_bir_lowering=False)

    # ---- DRAM I/O (per core) ----
    xT_d = nc.dram_tensor("xT", [D, S], F32, kind="ExternalInput")
    qwT_d = nc.dram_tensor("qwT", [D, D], F32, kind="ExternalInput")
    kwT_d = nc.dram_tensor("kwT", [D, D], F32, kind="ExternalInput")
    vwT_d = nc.dram_tensor("vwT", [D, D], F32, kind="ExternalInput")
    qb_d = nc.dram_tensor("qb", [D, 1], F32, kind="ExternalInput")
    kb_d = nc.dram_tensor("kb", [D, 1], F32, kind="ExternalInput")
    vb_d = nc.dram_tensor("vb", [D, 1], F32, kind="ExternalInput")
    AT_d = nc.dram_tensor("AT", [R, N], F32, kind="ExternalInput")       # A.T
    Bn_d = nc.dram_tensor("Bn", [R, D], F32, kind="ExternalInput")       # B natural
    btdwx_d = nc.dram_tensor("btdwx", [D, R + 1], F32, kind="ExternalInput")  # [B.T | dwx]
    btdwc_d = nc.dram_tensor("btdwc", [D, R + 1], F32, kind="ExternalInput")  # [B.T | dwc]
    dwb_d = nc.dram_tensor("dwb", [P, 1], F32, kind="ExternalInput")     # path bias diff

    neurons_d = nc.dram_tensor("neurons", [N, D], F32)                   # internal
    idxflat_d = nc.dram_tensor("idxflat", [S * K], U32)                  # internal

    sel_d = nc.dram_tensor("selected", [S * K, D], F32, kind="ExternalOutput")
    idx_d = nc.dram_tensor("topk_idx", [S, K], I32, kind="ExternalOutput")
    tkw_d = nc.dram_tensor("topk_w", [S, K], F32, kind="ExternalOutput")
    ctx_d = nc.dram_tensor("context", [S, D], F32, kind="ExternalOutput")

    with TileContext(nc) as tc:
        with tc.tile_pool(name="big", bufs=1) as big, \
             tc.tile_pool(name="projT", bufs=3) as projT, \
             tc.tile_pool(name="mid", bufs=2) as mid, \
             tc.tile_pool(name="est", bufs=4) as est, \
             tc.tile_pool(name="small", bufs=2) as small, \
             tc.tile_pool(name="gath", bufs=2) as gath, \
             tc.tile_pool(name="pp", bufs=3, space="PSUM") as pp, \
             tc.tile_pool(name="pl", bufs=3, space="PSUM") as pl, \
             tc.tile_pool(name="pc", bufs=2, space="PSUM") as pc:

            # ---------- persistent loads ----------
            xT = projT.tile([P, 8, S], F32, tag="projT")
            nc.sync.dma_start(out=xT[:, :, :], in_=xT_d.ap().rearrange("(c p) s -> p c s", p=P))
            AT = big.tile([R, N], F32, tag="AT")
            nc.sync.dma_start(out=AT[:, :], in_=AT_d.ap())
            Bn = mid.tile([R, D], F32, tag="m4k")
            nc.sync.dma_start(out=Bn[:, :], in_=Bn_d.ap())
            btdwx = big.tile([P, 8, R + 1], F32, tag="btdwx")
            nc.sync.dma_start(out=btdwx[:, :, :], in_=btdwx_d.ap().rearrange("(c p) r -> p c r", p=P))
            btdwc = big.tile([P, 8, R + 1], F32, tag="btdwc")
            nc.sync.dma_start(out=btdwc[:, :, :], in_=btdwc_d.ap().rearrange("(c p) r -> p c r", p=P))
            dwb = big.tile([P, 1], F32, tag="dwb")
            nc.sync.dma_start(out=dwb[:, :], in_=dwb_d.ap())
            biases = big.tile([P, 3, 8, 1], F32, tag="biases")
            for j, bd in enumerate((qb_d, kb_d, vb_d)):
                nc.sync.dma_start(out=biases[:, j, :, :], in_=bd.ap().rearrange("(c p) one -> p c one", p=P))

            ident = big.tile([P, P], F32, tag="ident")
            make_identity(nc, ident[:, :])

            # ---------- neurons table = A @ B -> DRAM ----------
            for nchunk in range(8):
                for blk in range(2):
                    psn = pp.tile([P, 512], F32, tag="pp")
                    nc.tensor.matmul(psn[:, :], AT[:, nchunk * P:(nchunk + 1) * P],
                                     Bn[:, blk * 512:(blk + 1) * 512], start=True, stop=True)
                    on = mid.tile([P, 512], F32, tag="m4k")
                    nc.vector.tensor_copy(on[:, :], psn[:, :])
                    nc.scalar.dma_start(
                        out=neurons_d.ap()[nchunk * P:(nchunk + 1) * P, blk * 512:(blk + 1) * 512],
                        in_=on[:, :])

            # ---------- projection helper (fp32, transposed layout) ----------
            def project(wd, jbias, outt):
                for dc in range(8):
                    wt = mid.tile([P, 8, P], F32, tag="m4k")
                    nc.sync.dma_start(
                        out=wt[:, :, :],
                        in_=wd.ap().rearrange("(c p) d -> p c d", p=P)[:, :, dc * P:(dc + 1) * P])
                    for sb in range(2):
                        psq = pp.tile([P, 512], F32, tag="pp")
                        for kc in range(8):
                            nc.tensor.matmul(psq[:, :], wt[:, kc, :],
                                             xT[:, kc, sb * 512:(sb + 1) * 512],
                                             start=(kc == 0), stop=(kc == 7))
                        nc.vector.tensor_scalar(
                            out=outt[:, dc, sb * 512:(sb + 1) * 512], in0=psq[:, :],
                            scalar1=biases[:, jbias, dc, :], scalar2=None, op0=ALU.add)

            # ---------- vT, then combo-rhs for all heads (frees vT) ----------
            vT = projT.tile([P, 8, S], F32, tag="projT")
            project(vwT_d, 2, vT)

            # crhs[h] = [v_h | z_h | g_h | ones]  (t x 98) for every head
            crhs = big.tile([P, 16, 8, 98], F32, tag="crhs")
            nc.vector.memset(crhs[:, :, :, 97:98], 1.0)
            for h in range(H):
                hc, hp = h // 2, (h % 2) * 64
                vslc = vT[hp:hp + 64, hc, :]
                for t in range(8):
                    pzg = pl.tile([P, R + 1], F32, tag="pl")
                    nc.tensor.matmul(pzg[:, :], vslc[:, t * P:(t + 1) * P],
                                     btdwc[hp:hp + 64, hc, :], start=True, stop=True)
                    ptr = pl.tile([P, 64], F32, tag="pl")
                    nc.tensor.transpose(ptr[:, :], vslc[:, t * P:(t + 1) * P],
                                        ident[hp:hp + 64, hp:hp + 64])
                    nc.vector.tensor_copy(crhs[:, h, t, 0:64], ptr[:, :])
                    nc.vector.tensor_copy(crhs[:, h, t, 64:97], pzg[:, :])

            # ---------- qT, kT ----------
            qT = projT.tile([P, 8, S], F32, tag="projT")
            project(qwT_d, 0, qT)
            kT = projT.tile([P, 8, S], F32, tag="projT")
            project(kwT_d, 1, kT)

            # ---------- u_t / d_x for all tokens ----------
            ut_sb = big.tile([P, 8, R + 1], F32, tag="ut_sb")
            for tcq in range(8):
                put = pl.tile([P, R + 1], F32, tag="pl")
                for kc in range(8):
                    nc.tensor.matmul(put[:, :], xT[:, kc, tcq * P:(tcq + 1) * P],
                                     btdwx[:, kc, :], start=(kc == 0), stop=(kc == 7))
                nc.vector.tensor_copy(ut_sb[:, tcq, :], put[:, :])

            # ---------- per s-quarter (256 tokens): attention -> scores -> top-16 -> gather ----------
            for sq in range(4):
                qsl = slice(sq * 256, (sq + 1) * 256)
                acc2 = small.tile([P, 2, R + 1], F32, tag="acc")
                nc.vector.memset(acc2[:, :, :], 0.0)
                for hc in range(8):  # head pair (rows 0-63 / 64-127 run concurrently)
                    esta = est.tile([P, 2, 8, P], F32, tag="estash")
                    estb = est.tile([P, 2, 8, P], F32, tag="estash")
                    for t in range(8):
                        for g2 in range(2):
                            hp = g2 * 64
                            psl = pl.tile([P, 256], F32, tag="pl")
                            nc.tensor.matmul(psl[:, :], kT[hp:hp + 64, hc, t * P:(t + 1) * P],
                                             qT[hp:hp + 64, hc, qsl], start=True, stop=True)
                            nc.scalar.activation(esta[:, g2, t, :], psl[:, 0:P], AF.Exp, scale=0.125)
                            nc.scalar.activation(estb[:, g2, t, :], psl[:, P:2 * P], AF.Exp, scale=0.125)
                    for sch, estx in ((0, esta), (1, estb)):
                        sc = sq * 2 + sch
                        ssl = slice(sc * P, (sc + 1) * P)
                        for g2 in range(2):
                            h = hc * 2 + g2
                            pcc = pc.tile([P, 98], F32, tag="pc")
                            for t in range(8):
                                nc.tensor.matmul(pcc[:, :], estx[:, g2, t, :],
                                                 crhs[:, h, t, :], start=(t == 0), stop=(t == 7))
                            # 1/Z with one Newton refinement
                            r0 = small.tile([P, 4], F32, tag="r0")
                            nc.vector.reciprocal(r0[:, 0:1], pcc[:, 97:98])
                            nc.vector.tensor_tensor(out=r0[:, 1:2], in0=pcc[:, 97:98], in1=r0[:, 0:1], op=ALU.mult)
                            nc.vector.tensor_scalar(out=r0[:, 2:3], in0=r0[:, 1:2],
                                                    scalar1=-1.0, scalar2=2.0, op0=ALU.mult, op1=ALU.add)
                            nc.vector.tensor_tensor(out=r0[:, 3:4], in0=r0[:, 0:1], in1=r0[:, 2:3], op=ALU.mult)
                            ctxt = small.tile([P, 64], F32, tag="ctxt")
                            nc.vector.tensor_scalar(out=ctxt[:, :], in0=pcc[:, 0:64],
                                                    scalar1=r0[:, 3:4], scalar2=None, op0=ALU.mult)
                            nc.scalar.dma_start(out=ctx_d.ap()[ssl, h * 64:(h + 1) * 64], in_=ctxt[:, :])
                            t33 = small.tile([P, R + 1], F32, tag="t33")
                            nc.vector.tensor_scalar(out=t33[:, :], in0=pcc[:, 64:97],
                                                    scalar1=r0[:, 3:4], scalar2=None, op0=ALU.mult)
                            nc.vector.tensor_add(acc2[:, sch, :], acc2[:, sch, :], t33[:, :])

                for sch in range(2):
                    sc = sq * 2 + sch
                    ssl = slice(sc * P, (sc + 1) * P)
                    # path weights + u
                    w0 = small.tile([P, 4], F32, tag="w0")
                    nc.vector.tensor_add(w0[:, 0:1], ut_sb[:, sc, R:R + 1], acc2[:, sch, R:R + 1])
                    nc.scalar.activation(w0[:, 1:2], w0[:, 0:1], AF.Sigmoid, bias=dwb[:, :])
                    nc.vector.tensor_scalar(out=w0[:, 2:3], in0=w0[:, 1:2],
                                            scalar1=-1.0, scalar2=1.0, op0=ALU.mult, op1=ALU.add)
                    u_ = small.tile([P, R], F32, tag="u_")
                    uc_ = small.tile([P, R], F32, tag="uc_")
                    nc.vector.tensor_scalar(out=u_[:, :], in0=ut_sb[:, sc, 0:R],
                                            scalar1=w0[:, 1:2], scalar2=None, op0=ALU.mult)
                    nc.vector.tensor_scalar(out=uc_[:, :], in0=acc2[:, sch, 0:R],
                                            scalar1=w0[:, 2:3], scalar2=None, op0=ALU.mult)
                    nc.vector.tensor_add(u_[:, :], u_[:, :], uc_[:, :])

                    # scores = u @ A.T (via PE transpose of u)
                    ptu = pl.tile([R, P], F32, tag="pl")
                    nc.tensor.transpose(ptu[:, :], u_[:, :], ident[:, :])
                    uT_ = small.tile([R, P], F32, tag="uT_")
                    nc.vector.tensor_copy(uT_[:, :], ptu[:, :])
                    scs = mid.tile([P, N], F32, tag="m4k")
                    for blk in range(2):
                        pss = pp.tile([P, 512], F32, tag="pp")
                        nc.tensor.matmul(pss[:, :], uT_[:, :],
                                         AT[:, blk * 512:(blk + 1) * 512], start=True, stop=True)
                        nc.vector.tensor_copy(scs[:, blk * 512:(blk + 1) * 512], pss[:, :])

                    # top-16 (two rounds of hw top-8)
                    vals = small.tile([P, K], F32, tag="vals")
                    ixs = small.tile([P, K], U32, tag="ixs")
                    nc.vector.max(out=vals[:, 0:8], in_=scs[:, :])
                    nc.vector.max_index(out=ixs[:, 0:8], in_max=vals[:, 0:8], in_values=scs[:, :])
                    scr = mid.tile([P, N], F32, tag="m4k")
                    nc.vector.match_replace(out=scr[:, :], in_to_replace=vals[:, 0:8],
                                            in_values=scs[:, :], imm_value=-1e30)
                    nc.vector.max(out=vals[:, 8:16], in_=scr[:, :])
                    nc.vector.max_index(out=ixs[:, 8:16], in_max=vals[:, 8:16], in_values=scr[:, :])

                    ix32 = small.tile([P, K], I32, tag="ix32")
                    nc.vector.tensor_copy(ix32[:, :], ixs[:, :])
                    nc.sync.dma_start(out=idx_d.ap()[ssl, :], in_=ix32[:, :])
                    e16 = small.tile([P, K + 4], F32, tag="e16")
                    nc.scalar.activation(e16[:, 0:K], vals[:, :], AF.Exp)
                    nc.vector.reduce_sum(out=e16[:, K:K + 1], in_=e16[:, 0:K], axis=mybir.AxisListType.X)
                    nc.vector.reciprocal(e16[:, K + 1:K + 2], e16[:, K:K + 1])
                    nc.vector.tensor_tensor(out=e16[:, K + 2:K + 3], in0=e16[:, K:K + 1],
                                            in1=e16[:, K + 1:K + 2], op=ALU.mult)
                    nc.vector.tensor_scalar(out=e16[:, K + 3:K + 4], in0=e16[:, K + 2:K + 3],
                                            scalar1=-1.0, scalar2=2.0, op0=ALU.mult, op1=ALU.add)
                    nc.vector.tensor_tensor(out=e16[:, K + 1:K + 2], in0=e16[:, K + 1:K + 2],
                                            in1=e16[:, K + 3:K + 4], op=ALU.mult)
                    wk = small.tile([P, K], F32, tag="wk")
                    nc.vector.tensor_scalar(out=wk[:, :], in0=e16[:, 0:K],
                                            scalar1=e16[:, K + 1:K + 2], scalar2=None, op0=ALU.mult)
                    nc.sync.dma_start(out=tkw_d.ap()[ssl, :], in_=wk[:, :])

                    # spill indices token-major; gather 16x128 neuron rows -> selected
                    nc.sync.dma_start(
                        out=idxflat_d.ap()[sc * P * K:(sc + 1) * P * K].rearrange("(p j) -> p j", j=K),
                        in_=ixs[:, :])
                    for gj in range(K):
                        g = sc * K + gj
                        idxcol = small.tile([P, 1], U32, tag="idxcol")
                        nc.sync.dma_start(
                            out=idxcol[:, :],
                            in_=idxflat_d.ap()[g * P:(g + 1) * P].rearrange("(p one) -> p one", one=1))
                        stg = gath.tile([P, D], F32, tag="stg")
                        nc.gpsimd.indirect_dma_start(
                            out=stg[:, :], out_offset=None,
                            in_=neurons_d.ap(),
                            in_offset=bass.IndirectOffsetOnAxis(ap=idxcol[:, :], axis=0))
                        nc.scalar.dma_start(out=sel_d.ap()[g * P:(g + 1) * P, :], in_=stg[:, :])

    nc.compile()
    return nc


def _host_prep(inputs):
    f32 = np.float32
    x = np.ascontiguousarray(np.asarray(inputs["x"], f32))
    q_w = np.asarray(inputs["q_w"], f32)
    k_w = np.asarray(inputs["k_w"], f32)
    v_w = np.asarray(inputs["v_w"], f32)
    A_ = np.asarray(inputs["neuron_A"], f32)
    B_ = np.asarray(inputs["neuron_B"], f32)
    pw = np.asarray(inputs["path_w"], f32)
    pb = np.asarray(inputs["path_b"], f32)

    common = {
        "qwT": np.ascontiguousarray(q_w.T),
        "kwT": np.ascontiguousarray(k_w.T),
        "vwT": np.ascontiguousarray(v_w.T),
        "qb": np.ascontiguousarray(np.asarray(inputs["q_b"], f32).reshape(D, 1)),
        "kb": np.ascontiguousarray(np.asarray(inputs["k_b"], f32).reshape(D, 1)),
        "vb": np.ascontiguousarray(np.asarray(inputs["v_b"], f32).reshape(D, 1)),
        "AT": np.ascontiguousarray(A_.T),
        "Bn": np.ascontiguousarray(B_),
        "btdwx": np.ascontiguousarray(
            np.concatenate([B_.T, (pw[0, :D] - pw[1, :D]).reshape(D, 1)], axis=1)),
        "btdwc": np.ascontiguousarray(
            np.concatenate([B_.T, (pw[0, D:] - pw[1, D:]).reshape(D, 1)], axis=1)),
        "dwb": np.full((P, 1), pb[0] - pb[1], f32),
    }
    in_maps = []
    for b in range(NCORES):
        m = dict(common)
        m["xT"] = np.ascontiguousarray(x[b].T)
        in_maps.append(m)
    return in_maps


def kernel(**inputs):
    if "nc" not in _CACHE:
        _CACHE["nc"] = _build()
    nc = _CACHE["nc"]
    in_maps = _host_prep(inputs)
    res = run_bass_kernel_spmd(nc, in_maps, list(range(NCORES))).results

    selected = np.stack([r["selected"].reshape(S, K, D) for r in res])
    topk_idx = np.stack([r["topk_idx"] for r in res]).astype(np.int32)
    topk_w = np.stack([r["topk_w"] for r in res])
    context = np.stack([r["context"] for r in res])
    return selected, topk_idx, topk_w, context


# revision 19
# speedup vs baseline: 1.0980x; 1.0980x over previous
"""NeuronRouter kernel for 8x Trainium2 NeuronCores.

Problem (hardcoded shapes): B=8, S=1024, D=1024, H=16, DH=64, N=1024, R=32, K=16.

Sharding: data-parallel over batch B — core b computes batch element b.
Everything else (weights, neuron pool) is replicated.

All score-path matmuls run in native fp32 (4 cyc/row) so the top-k selection
matches a pure-fp32 reference exactly; outputs are fp32-accurate.

Structure (v2): s-chunk pipelined. After the projections, each 128-token
s-chunk flows through attention -> path-combine -> scores -> top-16 ->
selected-row gather independently, so the (DMA-heavy) gather of chunk i
overlaps the compute of chunks i+1.. and the serial tail is one chunk long.
"""

import numpy as np

import concourse.bass as bass
import concourse.mybir as mybir
from concourse import bacc
from concourse.tile import TileContext
from concourse.bass_utils import run_bass_kernel_spmd
from concourse.masks import make_identity

F32 = mybir.dt.float32
U32 = mybir.dt.uint32
I32 = mybir.dt.int32
AF = mybir.ActivationFunctionType
ALU = mybir.AluOpType

B, S, D = 8, 1024, 1024
H, DH = 16, 64
N, R, K = 1024, 32, 16
P = 128
NCORES = 8

_CACHE = {}


def _build():
    nc = bacc.Bacc(target_bir_lowering=False)

    # ---- DRAM I/O (per core) ----
    xT_d = nc.dram_tensor("xT", [D, S], F32, kind="ExternalInput")
    qwT_d = nc.dram_tensor("qwT", [D, D], F32, kind="ExternalInput")
    kwT_d = nc.dram_tensor("kwT", [D, D], F32, kind="ExternalInput")
    vwT_d = nc.dram_tensor("vwT", [D, D], F32, kind="ExternalInput")
    qb_d = nc.dram_tensor("qb", [D, 1], F32, kind="ExternalInput")
    kb_d = nc.dram_tensor("kb", [D, 1], F32, kind="ExternalInput")
    vb_d = nc.dram_tensor("vb", [D, 1], F32, kind="ExternalInput")
    AT_d = nc.dram_tensor("AT", [R, N], F32, kind="ExternalInput")       # A.T
    Bn_d = nc.dram_tensor("Bn", [R, D], F32, kind="ExternalInput")       # B natural
    btdwx_d = nc.dram_tensor("btdwx", [D, R + 1], F32, kind="ExternalInput")  # [B.T | dwx]
    btdwc_d = nc.dram_tensor("btdwc", [D, R + 1], F32, kind="ExternalInput")  # [B.T | dwc]
    dwb_d = nc.dram_tensor("dwb", [P, 1], F32, kind="ExternalInput")     # path bias diff

    neurons_d = nc.dram_tensor("neurons", [N, D], F32)                   # internal
    idxflat_d = nc.dram_tensor("idxflat", [S * K], U32)                  # internal

    sel_d = nc.dram_tensor("selected", [S * K, D], F32, kind="ExternalOutput")
    idx_d = nc.dram_tensor("topk_idx", [S, K], I32, kind="ExternalOutput")
    tkw_d = nc.dram_tensor("topk_w", [S, K], F32, kind="ExternalOutput")
    ctx_d = nc.dram_tensor("context", [S, D], F32, kind="ExternalOutput")

    with TileContext(nc) as tc:
        with tc.tile_pool(name="big", bufs=1) as big, \
             tc.tile_pool(name="projT", bufs=3) as projT, \
             tc.tile_pool(name="mid", bufs=2) as mid, \
             tc.tile_pool(name="est", bufs=1) as est, \
             tc.tile_pool(name="small", bufs=2) as small, \
             tc.tile_pool(name="gath", bufs=3) as gath, \
             tc.tile_pool(name="pb", bufs=3, space="PSUM") as pb, \
             tc.tile_pool(name="pc", bufs=2, space="PSUM") as pc:

            # ---------- persistent loads ----------
            xT = projT.tile([P, 8, S], F32, tag="projT")
            nc.sync.dma_start(out=xT[:, :, :], in_=xT_d.ap().rearrange("(c p) s -> p c s", p=P))
            AT = big.tile([R, N], F32, tag="AT")
            nc.sync.dma_start(out=AT[:, :], in_=AT_d.ap())
            Bn = mid.tile([R, D], F32, tag="m4k")
            nc.sync.dma_start(out=Bn[:, :], in_=Bn_d.ap())
            btdwx = big.tile([P, 8, R + 1], F32, tag="btdwx")
            nc.sync.dma_start(out=btdwx[:, :, :], in_=btdwx_d.ap().rearrange("(c p) r -> p c r", p=P))
            btdwc = big.tile([P, 8, R + 1], F32, tag="btdwc")
            nc.sync.dma_start(out=btdwc[:, :, :], in_=btdwc_d.ap().rearrange("(c p) r -> p c r", p=P))
            dwb = big.tile([P, 1], F32, tag="dwb")
            nc.sync.dma_start(out=dwb[:, :], in_=dwb_d.ap())
            biases = big.tile([P, 3, 8, 1], F32, tag="biases")
            for j, bd in enumerate((qb_d, kb_d, vb_d)):
                nc.sync.dma_start(out=biases[:, j, :, :], in_=bd.ap().rearrange("(c p) one -> p c one", p=P))

            ident = big.tile([P, P], F32, tag="ident")
            make_identity(nc, ident[:, :])

            # ---------- neurons table = A @ B -> DRAM ----------
            for nchunk in range(8):
                for blk in range(2):
                    psn = pb.tile([P, 512], F32, tag="pb")
                    nc.tensor.matmul(psn[:, :], AT[:, nchunk * P:(nchunk + 1) * P],
                                     Bn[:, blk * 512:(blk + 1) * 512], start=True, stop=True)
                    on = mid.tile([P, 512], F32, tag="m4k")
                    nc.vector.tensor_copy(on[:, :], psn[:, :])
                    nc.scalar.dma_start(
                        out=neurons_d.ap()[nchunk * P:(nchunk + 1) * P, blk * 512:(blk + 1) * 512],
                        in_=on[:, :])

            # ---------- projection helper (fp32, transposed layout) ----------
            def project(wd, jbias, outt):
                for dc in range(8):
                    wt = mid.tile([P, 8, P], F32, tag="m4k")
                    nc.sync.dma_start(
                        out=wt[:, :, :],
                        in_=wd.ap().rearrange("(c p) d -> p c d", p=P)[:, :, dc * P:(dc + 1) * P])
                    for sb in range(2):
                        psq = pb.tile([P, 512], F32, tag="pb")
                        for kc in range(8):
                            nc.tensor.matmul(psq[:, :], wt[:, kc, :],
                                             xT[:, kc, sb * 512:(sb + 1) * 512],
                                             start=(kc == 0), stop=(kc == 7))
                        nc.vector.tensor_scalar(
                            out=outt[:, dc, sb * 512:(sb + 1) * 512], in0=psq[:, :],
                            scalar1=biases[:, jbias, dc, :], scalar2=None, op0=ALU.add)

            # ---------- vT, then combo-rhs for all heads (frees vT) ----------
            vT = projT.tile([P, 8, S], F32, tag="projT")
            project(vwT_d, 2, vT)

            # crhs[h] = [v_h | z_h | g_h | ones]  (t x 98) for every head
            crhs = big.tile([P, 16, 8, 98], F32, tag="crhs")
            nc.vector.memset(crhs[:, :, :, 97:98], 1.0)
            for h in range(H):
                hc, hp = h // 2, (h % 2) * 64
                vslc = vT[hp:hp + 64, hc, :]
                for t in range(8):
                    pzg = pb.tile([P, R + 1], F32, tag="pb")
                    nc.tensor.matmul(pzg[:, :], vslc[:, t * P:(t + 1) * P],
                                     btdwc[hp:hp + 64, hc, :], start=True, stop=True)
                    ptr = pb.tile([P, 64], F32, tag="pb")
                    nc.tensor.transpose(ptr[:, :], vslc[:, t * P:(t + 1) * P],
                                        ident[hp:hp + 64, hp:hp + 64])
                    nc.vector.tensor_copy(crhs[:, h, t, 0:64], ptr[:, :])
                    nc.vector.tensor_copy(crhs[:, h, t, 64:97], pzg[:, :])

            # ---------- qT, kT ----------
            qT = projT.tile([P, 8, S], F32, tag="projT")
            project(qwT_d, 0, qT)
            kT = projT.tile([P, 8, S], F32, tag="projT")
            project(kwT_d, 1, kT)

            # ---------- u_t / d_x for all tokens ----------
            ut_sb = big.tile([P, 8, R + 1], F32, tag="ut_sb")
            for tcq in range(8):
                put = pb.tile([P, R + 1], F32, tag="pb")
                for kc in range(8):
                    nc.tensor.matmul(put[:, :], xT[:, kc, tcq * P:(tcq + 1) * P],
                                     btdwx[:, kc, :], start=(kc == 0), stop=(kc == 7))
                nc.vector.tensor_copy(ut_sb[:, tcq, :], put[:, :])

            # ---------- per s-quarter (256 tokens): attention -> scores -> top-16 -> gather ----------
            for sq in range(4):
                qsl = slice(sq * 256, (sq + 1) * 256)
                acc2 = small.tile([P, 2, R + 1], F32, tag="acc")
                nc.vector.memset(acc2[:, :, :], 0.0)
                for hc in range(8):  # head pair (rows 0-63 / 64-127 run concurrently)
                    estash2 = est.tile([P, 8, 2, 256], F32, tag="estash")
                    for t in range(8):
                        psl = pb.tile([P, 2, 512], F32, tag="pb")
                        nc.tensor.matmul(psl[:, 0, 0:256], kT[0:64, hc, t * P:(t + 1) * P],
                                         qT[0:64, hc, qsl], start=True, stop=True)
                        nc.tensor.matmul(psl[:, 1, 0:256], kT[64:128, hc, t * P:(t + 1) * P],
                                         qT[64:128, hc, qsl], start=True, stop=True)
                        nc.scalar.activation(estash2[:, t, :, :], psl[:, :, 0:256],
                                             AF.Exp, scale=0.125)
                    for sch in range(2):
                        sc = sq * 2 + sch
                        ssl = slice(sc * P, (sc + 1) * P)
                        for g2 in range(2):
                            h = hc * 2 + g2
                            pcc = pc.tile([P, 98], F32, tag="pc")
                            for t in range(8):
                                nc.tensor.matmul(pcc[:, :], estash2[:, t, g2, sch * P:(sch + 1) * P],
                                                 crhs[:, h, t, :], start=(t == 0), stop=(t == 7))
                            # 1/Z with one Newton refinement
                            r0 = small.tile([P, 4], F32, tag="r0")
                            nc.vector.reciprocal(r0[:, 0:1], pcc[:, 97:98])
                            nc.vector.tensor_tensor(out=r0[:, 1:2], in0=pcc[:, 97:98], in1=r0[:, 0:1], op=ALU.mult)
                            nc.vector.tensor_scalar(out=r0[:, 2:3], in0=r0[:, 1:2],
                                                    scalar1=-1.0, scalar2=2.0, op0=ALU.mult, op1=ALU.add)
                            nc.vector.tensor_tensor(out=r0[:, 3:4], in0=r0[:, 0:1], in1=r0[:, 2:3], op=ALU.mult)
                            ctxt = small.tile([P, 64], F32, tag="ctxt")
                            nc.vector.tensor_scalar(out=ctxt[:, :], in0=pcc[:, 0:64],
                                                    scalar1=r0[:, 3:4], scalar2=None, op0=ALU.mult)
                            nc.scalar.dma_start(out=ctx_d.ap()[ssl, h * 64:(h + 1) * 64], in_=ctxt[:, :])
                            t33 = small.tile([P, R + 1], F32, tag="t33")
                            nc.vector.tensor_scalar(out=t33[:, :], in0=pcc[:, 64:97],
                                                    scalar1=r0[:, 3:4], scalar2=None, op0=ALU.mult)
                            nc.vector.tensor_add(acc2[:, sch, :], acc2[:, sch, :], t33[:, :])

                for sch in range(2):
                    sc = sq * 2 + sch
                    ssl = slice(sc * P, (sc + 1) * P)
                    # path weights + u
                    w0 = small.tile([P, 4], F32, tag="w0")
                    nc.vector.tensor_add(w0[:, 0:1], ut_sb[:, sc, R:R + 1], acc2[:, sch, R:R + 1])
                    nc.scalar.activation(w0[:, 1:2], w0[:, 0:1], AF.Sigmoid, bias=dwb[:, :])
                    nc.vector.tensor_scalar(out=w0[:, 2:3], in0=w0[:, 1:2],
                                            scalar1=-1.0, scalar2=1.0, op0=ALU.mult, op1=ALU.add)
                    u_ = small.tile([P, R], F32, tag="u_")
                    uc_ = small.tile([P, R], F32, tag="uc_")
                    nc.vector.tensor_scalar(out=u_[:, :], in0=ut_sb[:, sc, 0:R],
                                            scalar1=w0[:, 1:2], scalar2=None, op0=ALU.mult)
                    nc.vector.tensor_scalar(out=uc_[:, :], in0=acc2[:, sch, 0:R],
                                            scalar1=w0[:, 2:3], scalar2=None, op0=ALU.mult)
                    nc.vector.tensor_add(u_[:, :], u_[:, :], uc_[:, :])

                    # scores = u @ A.T (via PE transpose of u)
                    ptu = pb.tile([R, P], F32, tag="pb")
                    nc.tensor.transpose(ptu[:, :], u_[:, :], ident[:, :])
                    uT_ = small.tile([R, P], F32, tag="uT_")
                    nc.vector.tensor_copy(uT_[:, :], ptu[:, :])
                    scs = mid.tile([P, N], F32, tag="m4k")
                    for blk in range(2):
                        pss = pb.tile([P, 512], F32, tag="pb")
                        nc.tensor.matmul(pss[:, :], uT_[:, :],
                                         AT[:, blk * 512:(blk + 1) * 512], start=True, stop=True)
                        nc.vector.tensor_copy(scs[:, blk * 512:(blk + 1) * 512], pss[:, :])

                    # top-16 (two rounds of hw top-8)
                    vals = small.tile([P, K], F32, tag="vals")
                    ixs = small.tile([P, K], U32, tag="ixs")
                    nc.vector.max(out=vals[:, 0:8], in_=scs[:, :])
                    nc.vector.max_index(out=ixs[:, 0:8], in_max=vals[:, 0:8], in_values=scs[:, :])
                    scr = mid.tile([P, N], F32, tag="m4k")
                    nc.vector.match_replace(out=scr[:, :], in_to_replace=vals[:, 0:8],
                                            in_values=scs[:, :], imm_value=-1e30)
                    nc.vector.max(out=vals[:, 8:16], in_=scr[:, :])
                    nc.vector.max_index(out=ixs[:, 8:16], in_max=vals[:, 8:16], in_values=scr[:, :])

                    ix32 = small.tile([P, K], I32, tag="ix32")
                    nc.vector.tensor_copy(ix32[:, :], ixs[:, :])
                    nc.sync.dma_start(out=idx_d.ap()[ssl, :], in_=ix32[:, :])
                    e16 = small.tile([P, K + 4], F32, tag="e16")
                    nc.scalar.activation(e16[:, 0:K], vals[:, :], AF.Exp)
                    nc.vector.reduce_sum(out=e16[:, K:K + 1], in_=e16[:, 0:K], axis=mybir.AxisListType.X)
                    nc.vector.reciprocal(e16[:, K + 1:K + 2], e16[:, K:K + 1])
                    nc.vector.tensor_tensor(out=e16[:, K + 2:K + 3], in0=e16[:, K:K + 1],
                                            in1=e16[:, K + 1:K + 2], op=ALU.mult)
                    nc.vector.tensor_scalar(out=e16[:, K + 3:K + 4], in0=e16[:, K + 2:K + 3],
                                            scalar1=-1.0, scalar2=2.0, op0=ALU.mult, op1=ALU.add)
                    nc.vector.tensor_tensor(out=e16[:, K + 1:K + 2], in0=e16[:, K + 1:K + 2],
                                            in1=e16[:, K + 3:K + 4], op=ALU.mult)
                    wk = small.tile([P, K], F32, tag="wk")
                    nc.vector.tensor_scalar(out=wk[:, :], in0=e16[:, 0:K],
                                            scalar1=e16[:, K + 1:K + 2], scalar2=None, op0=ALU.mult)
                    nc.sync.dma_start(out=tkw_d.ap()[ssl, :], in_=wk[:, :])

                    # spill indices token-major; gather 16x128 neuron rows -> selected
                    nc.sync.dma_start(
                        out=idxflat_d.ap()[sc * P * K:(sc + 1) * P * K].rearrange("(p j) -> p j", j=K),
                        in_=ixs[:, :])
                    for gj in range(K):
                        g = sc * K + gj
                        idxcol = small.tile([P, 1], U32, tag="idxcol")
                        nc.sync.dma_start(
                            out=idxcol[:, :],
                            in_=idxflat_d.ap()[g * P:(g + 1) * P].rearrange("(p one) -> p one", one=1))
                        stg = gath.tile([P, D], F32, tag="stg")
                        nc.gpsimd.indirect_dma_start(
                            out=stg[:, :], out_offset=None,
                            in_=neurons_d.ap(),
                            in_offset=bass.IndirectOffsetOnAxis(ap=idxcol[:, :], axis=0))
                        nc.scalar.dma_start(out=sel_d.ap()[g * P:(g + 1) * P, :], in_=stg[:, :])

    nc.compile()
    return nc


def _host_prep(inputs):
    f32 = np.float32
    x = np.ascontiguousarray(np.asarray(inputs["x"], f32))
    q_w = np.asarray(inputs["q_w"], f32)
    k_w = np.asarray(inputs["k_w"], f32)
    v_w = np.asarray(inputs["v_w"], f32)
    A_ = np.asarray(inputs["neuron_A"], f32)
    B_ = np.asarray(inputs["neuron_B"], f32)
    pw = np.asarray(inputs["path_w"], f32)
    pb = np.asarray(inputs["path_b"], f32)

    common = {
        "qwT": np.ascontiguousarray(q_w.T),
        "kwT": np.ascontiguousarray(k_w.T),
        "vwT": np.ascontiguousarray(v_w.T),
        "qb": np.ascontiguousarray(np.asarray(inputs["q_b"], f32).reshape(D, 1)),
        "kb": np.ascontiguousarray(np.asarray(inputs["k_b"], f32).reshape(D, 1)),
        "vb": np.ascontiguousarray(np.asarray(inputs["v_b"], f32).reshape(D, 1)),
        "AT": np.ascontiguousarray(A_.T),
        "Bn": np.ascontiguousarray(B_),
        "btdwx": np.ascontiguousarray(
            np.concatenate([B_.T, (pw[0, :D] - pw[1, :D]).reshape(D, 1)], axis=1)),
        "btdwc": np.ascontiguousarray(
            np.concatenate([B_.T, (pw[0, D:] - pw[1, D:]).reshape(D, 1)], axis=1)),
        "dwb": np.full((P, 1), pb[0] - pb[1], f32),
    }
    in_maps = []
    for b in range(NCORES):
        m = dict(common)
        m["xT"] = np.ascontiguousarray(x[b].T)
        in_maps.append(m)
    return in_maps


def kernel(**inputs):
    if "nc" not in _CACHE:
        _CACHE["nc"] = _build()
    nc = _CACHE["nc"]
    in_maps = _host_prep(inputs)
    res = run_bass_kernel_spmd(nc, in_maps, list(range(NCORES))).results

    selected = np.stack([r["selected"].reshape(S, K, D) for r in res])
    topk_idx = np.stack([r["topk_idx"] for r in res]).astype(np.int32)
    topk_w = np.stack([r["topk_w"] for r in res])
    context = np.stack([r["context"] for r in res])
    return selected, topk_idx, topk_w, context
